# revision 1
# baseline (speedup 1.0000x reference)
"""CirLinear Trainium2 kernel: y = x @ build_weight(W, alphas, gumbels)^T + bias.

Strategy (8 NeuronCores, no collectives), 2x4 grid:
 - core c = tshard*4 + oshard: tokens [8192*tshard, +8192), out rows
   [512*oshard, +512)
 - circulant weight build done locally per core (512 rows, two 256-row
   chunks pipelined with the matmul)
 - x is passed host-transposed (xT slice [2048, 8192] f32) so the
   contraction dim lands on SBUF partitions with contiguous DMA; the
   f32->bf16 cast happens inside the load DMA (SWDGE)
 - bf16 matmul (lhsT = wT slice [128i,128o], rhs = xT tile [128i,512t])
   with fp32 PSUM accumulation over 16 K-chunks, bias added on the
   scalar engine, fp32 output out^T [512, 8192]
 - host assembles the 2x4 grid and transposes back
"""
import sys

sys.path.insert(0, '/opt/trn_rl_repo')

import numpy as np

import concourse.bass as bass
from concourse import bacc
import concourse.mybir as mybir
from concourse.tile import TileContext
from concourse.bass_utils import run_bass_kernel_spmd

N_CORES = 8
T_SHARDS, O_SHARDS = 2, 4
BATCH, TOKENS, IN_F, OUT_F = 16, 1024, 2048, 2048
TOK_TOTAL = BATCH * TOKENS            # 16384
TOK = TOK_TOTAL // T_SHARDS           # 8192 tokens per core
ROWS = OUT_F // O_SHARDS              # 512 out-features per core
N_CH = ROWS // 256                    # 2 build chunks of 256 rows
SCALES = [2, 4, 8, 16, 32, 64]
N_IC = IN_F // 128                    # 16 contraction chunks
N_TG = TOK // 512                     # 16 token groups of 512
N_OS = ROWS // 128                    # 4 output-row subtiles

bf16 = mybir.dt.bfloat16
f32 = mybir.dt.float32

_CACHE = {}


def _build_nc():
    nc = bacc.Bacc("TRN2", target_bir_lowering=False, debug=False, num_devices=N_CORES)
    xT = nc.dram_tensor("xT", [IN_F, TOK], bf16, kind="ExternalInput")
    ws = nc.dram_tensor("ws", [ROWS, IN_F], bf16, kind="ExternalInput")
    bias_s = nc.dram_tensor("bias_s", [1, ROWS], f32, kind="ExternalInput")
    alphas = nc.dram_tensor("alphas", [1, 7], f32, kind="ExternalInput")
    gumbels = nc.dram_tensor("gumbels", [1, 7], f32, kind="ExternalInput")
    out = nc.dram_tensor("out", [ROWS, TOK], f32, kind="ExternalOutput")

    w_loc = nc.dram_tensor("w_loc", [ROWS, IN_F], bf16)

    with TileContext(nc) as tc:
        # ---------- softmax(alphas + gumbels) broadcast to 128 partitions ----------
        asb = nc.alloc_sbuf_tensor("asb", [128, 7], f32).ap()
        gsb = nc.alloc_sbuf_tensor("gsb", [128, 7], f32).ap()
        a_bc = nc.alloc_sbuf_tensor("a_bc", [128, 7], f32).ap()
        ssum = nc.alloc_sbuf_tensor("ssum", [128, 1], f32).ap()
        nc.gpsimd.dma_start(out=asb, in_=bass.AP(tensor=alphas, offset=0, ap=[[0, 128], [1, 7]]))
        nc.gpsimd.dma_start(out=gsb, in_=bass.AP(tensor=gumbels, offset=0, ap=[[0, 128], [1, 7]]))
        nc.vector.tensor_tensor(out=asb, in0=asb, in1=gsb, op=mybir.AluOpType.add)
        nc.scalar.activation(out=asb, in_=asb, func=mybir.ActivationFunctionType.Exp)
        nc.vector.tensor_reduce(out=ssum, in_=asb, axis=mybir.AxisListType.X, op=mybir.AluOpType.add)
        nc.vector.reciprocal(out=ssum, in_=ssum)
        nc.vector.tensor_scalar_mul(a_bc, asb, ssum)

        # ---------- bias: [1, 512] -> [128 part, 4] (per-osub per-partition) ----------
        bias_sb = nc.alloc_sbuf_tensor("bias_sb", [128, N_OS], f32).ap()
        with nc.allow_non_contiguous_dma(reason="512-element one-time bias transpose"):
            nc.gpsimd.dma_start(out=bias_sb, in_=bass.AP(tensor=bias_s, offset=0, ap=[[1, 128], [128, N_OS]]))

        # ---------- circulant weight build: 2 chunks of 256 rows ----------
        # chunk partition = (q64, p64) : 4*32 = 128 ; free = (r64, s64) : 64*64
        wb = nc.alloc_sbuf_tensor("wb", [128, 4096], bf16).ap()
        acc = nc.alloc_sbuf_tensor("acc", [128, 4096], f32).ap()
        # double-buffered per scale parity so ACT pads / DVE reduces of
        # consecutive scales don't WAR-serialize on shared buffers
        wbpad2 = [nc.alloc_sbuf_tensor(f"wbpad{i}", [128, 8192], bf16).ap() for i in range(2)]
        d_raw2 = [nc.alloc_sbuf_tensor(f"d_raw{i}", [128, 2048], f32).ap() for i in range(2)]
        dpad2 = [nc.alloc_sbuf_tensor(f"dpad{i}", [128, 4096], bf16).ap() for i in range(2)]
        ws_4d = ws.ap().rearrange("(q r) (p s) -> q p r s", r=64, s=64)
        wloc_4d = w_loc.ap().rearrange("(q r) (p s) -> q p r s", r=64, s=64)

        wT = [nc.alloc_sbuf_tensor(f"wT{ic}", [128, ROWS], bf16).ap() for ic in range(N_IC)]

        def sb(t, off, dims):
            return bass.AP(tensor=t.tensor, offset=off, ap=[list(t.ap[0])] + dims)

        for ch in range(N_CH):
            for q in range(4):
                nc.sync.dma_start(out=wb[q * 32:(q + 1) * 32, :], in_=ws_4d[ch * 4 + q])
            nc.vector.tensor_scalar_mul(acc, wb, a_bc[:, 0:1])
            for idx, b in enumerate(SCALES, start=1):
                nv = 64 // b
                wbpad, d_raw, dpad = wbpad2[idx % 2], d_raw2[idx % 2], dpad2[idx % 2]
                src = sb(wb, 0, [[64, 64], [b, nv], [1, b]])
                for half in range(2):
                    dst = sb(wbpad, half * b, [[128, 64], [2 * b, nv], [1, b]])
                    if half == 0:
                        nc.scalar.copy(out=dst, in_=src)
                    else:
                        nc.vector.tensor_copy(out=dst, in_=src)
                # diagonal reduce: split the instruction over the smaller of u/k
                if nv <= b:
                    for u in range(nv):
                        rin = sb(wbpad, u * b * 128, [[2 * b, nv], [1, b], [129, b]])
                        rout = sb(d_raw, u * 64, [[b, nv], [1, b]])
                        nc.vector.tensor_reduce(out=rout, in_=rin, axis=mybir.AxisListType.X,
                                                op=mybir.AluOpType.add)
                else:
                    for k in range(b):
                        rin = sb(wbpad, k, [[b * 128, nv], [2 * b, nv], [129, b]])
                        rout = sb(d_raw, k, [[64, nv], [b, nv]])
                        nc.vector.tensor_reduce(out=rout, in_=rin, axis=mybir.AxisListType.X,
                                                op=mybir.AluOpType.add)
                dsrc = sb(d_raw, 0, [[64, nv], [b, nv], [1, b]])
                for half in range(2):
                    ddst = sb(dpad, half * b, [[128, nv], [2 * b, nv], [1, b]])
                    nc.vector.tensor_scalar(out=ddst, in0=dsrc, scalar1=a_bc[:, idx:idx + 1],
                                            scalar2=1.0 / b, op0=mybir.AluOpType.mult,
                                            op1=mybir.AluOpType.mult)
                # expand: split over the smaller of u/r
                if nv <= b:
                    for u in range(nv):
                        aout = sb(acc, u * b * 64, [[b, nv], [64, b], [1, b]])
                        din = sb(dpad, u * 128 + b, [[2 * b, nv], [-1, b], [1, b]])
                        nc.vector.tensor_tensor(out=aout, in0=aout, in1=din, op=mybir.AluOpType.add)
                else:
                    for r in range(b):
                        aout = sb(acc, r * 64, [[b * 64, nv], [b, nv], [1, b]])
                        din = sb(dpad, b - r, [[128, nv], [2 * b, nv], [1, b]])
                        nc.vector.tensor_tensor(out=aout, in0=aout, in1=din, op=mybir.AluOpType.add)
            # scatter chunk -> w_loc rows [256ch, 256ch+256) (bf16, SWDGE cast)
            for q in range(4):
                nc.gpsimd.dma_start(out=wloc_4d[ch * 4 + q], in_=acc[q * 32:(q + 1) * 32, :])
            # transposed reload of this chunk's columns into wT
            for ic in range(N_IC):
                nc.sync.dma_start(out=wT[ic][:, ch * 256:(ch + 1) * 256],
                                  in_=w_loc.ap()[ch * 256:(ch + 1) * 256, ic * 128:(ic + 1) * 128],
                                  transpose=True)

        # ---------- main matmul over 16 token groups ----------
        with (
            tc.tile_pool(name="xt", bufs=3) as xt_pool,
            tc.tile_pool(name="psum", bufs=8, space="PSUM") as psum_pool,
            tc.tile_pool(name="osb", bufs=8) as osb_pool,
        ):
            for tg in range(N_TG):
                # one SWDGE cast-DMA loads all 16 ic-chunks for this token group
                xt = xt_pool.tile([128, N_IC * 512], bf16, name="xt")
                nc.sync.dma_start(
                    out=xt[:],
                    in_=bass.AP(tensor=xT, offset=tg * 512,
                                ap=[[TOK, 128], [128 * TOK, N_IC], [1, 512]]))
                psums = [psum_pool.tile([128, 512], f32, name=f"ps{o}", tag="ps")
                         for o in range(N_OS)]
                for ic in range(N_IC):
                    rhs = xt[:, ic * 512:(ic + 1) * 512]
                    for o in range(N_OS):
                        nc.tensor.matmul(psums[o][:], wT[ic][:, o * 128:(o + 1) * 128], rhs,
                                         start=(ic == 0), stop=(ic == N_IC - 1))
                for o in range(N_OS):
                    ot = osb_pool.tile([128, 512], f32, name="ot")
                    nc.scalar.activation(out=ot[:], in_=psums[o][:],
                                         func=mybir.ActivationFunctionType.Identity,
                                         bias=bias_sb[:, o:o + 1], scale=1.0)
                    nc.sync.dma_start(out=out.ap()[o * 128:(o + 1) * 128, tg * 512:(tg + 1) * 512],
                                      in_=ot[:])

    nc.compile()
    return nc


def make_in_maps(x, weight, bias, alphas, gumbels):
    import ml_dtypes
    x2 = np.asarray(x, np.float32).reshape(TOK_TOTAL, IN_F)
    xTh = np.ascontiguousarray(x2.T).astype(ml_dtypes.bfloat16)   # [2048, 16384]
    xslices = [np.ascontiguousarray(xTh[:, t * TOK:(t + 1) * TOK]) for t in range(T_SHARDS)]
    weight = np.asarray(weight, np.float32)
    bias = np.asarray(bias, np.float32)
    wslices = [np.ascontiguousarray(weight[o * ROWS:(o + 1) * ROWS]).astype(ml_dtypes.bfloat16)
               for o in range(O_SHARDS)]
    bslices = [np.ascontiguousarray(bias[o * ROWS:(o + 1) * ROWS]).reshape(1, ROWS)
               for o in range(O_SHARDS)]
    al = np.asarray(alphas, np.float32).reshape(1, 7)
    gu = np.asarray(gumbels, np.float32).reshape(1, 7)
    in_maps = []
    for c in range(N_CORES):
        t, o = divmod(c, O_SHARDS)
        in_maps.append({"xT": xslices[t], "ws": wslices[o], "bias_s": bslices[o],
                        "alphas": al, "gumbels": gu})
    return in_maps


def kernel(x, weight, bias, alphas, gumbels):
    if "nc" not in _CACHE:
        _CACHE["nc"] = _build_nc()
    nc = _CACHE["nc"]
    in_maps = make_in_maps(x, weight, bias, alphas, gumbels)
    res = run_bass_kernel_spmd(nc, in_maps, core_ids=list(range(N_CORES)))
    # assemble: rows = o-shards, cols = t-shards
    row_blocks = []
    for o in range(O_SHARDS):
        row_blocks.append(np.concatenate(
            [res.results[t * O_SHARDS + o]["out"] for t in range(T_SHARDS)], axis=1))
    full_t = np.concatenate(row_blocks, axis=0)              # [2048, 16384]
    return np.ascontiguousarray(full_t.T).reshape(BATCH, TOKENS, OUT_F)



# revision 6
# speedup vs baseline: 1.0560x; 1.0560x over previous
"""CirLinear Trainium2 kernel: y = x @ build_weight(W, alphas, gumbels)^T + bias.

Strategy (8 NeuronCores, no collectives), 2x4 grid:
 - core c = tshard*4 + oshard: tokens [8192*tshard, +8192), out rows
   [512*oshard, +512)
 - weight build via hierarchical straight-diagonal pyramid:
   * up-sweep: straight diag partial sums s_b/t_b compose 2x per level
     (s_2b = s_b(00)+s_b(11)+t_b(01) etc), so cyclic diag sums for all 6
     scales cost ~2 passes over the data instead of one pass per scale
   * down-sweep: per-scale contributions pushed down a signed-lag pyramid
     G_b[U,V,lam], lam = s-r in (-b,b); parent lag = lam + b*(Vpar-Upar)
     needs no wrap handling
   * layout: partition = (P in-col-pair 16, q out-block 8); free =
     (r, phat, s); all scales operate free-dim only
 - acc (bf16 W_eff chunk) -> flat DRAM store (contiguous per partition,
   4 pipelined groups) -> 16 HW DMA-transposes [512,128] -> wT[ic] matmul
   lhsT tiles; no descriptor-heavy scatter, no big serial DVE phase
 - bf16 matmul (lhsT = wT[ic][:,128o:+128], rhs = xT tile [128i,512t])
   with fp32 PSUM accumulation over 16 K-chunks, bias added on the
   scalar engine, bf16 output out^T [512, 8192], host casts to f32
"""
import sys

sys.path.insert(0, '/opt/trn_rl_repo')

import numpy as np

import concourse.bass as bass
from concourse import bacc
import concourse.mybir as mybir
from concourse.tile import TileContext
from concourse.bass_utils import run_bass_kernel_spmd

N_CORES = 8
T_SHARDS, O_SHARDS = 2, 4
BATCH, TOKENS, IN_F, OUT_F = 16, 1024, 2048, 2048
TOK_TOTAL = BATCH * TOKENS            # 16384
TOK = TOK_TOTAL // T_SHARDS           # 8192 tokens per core
ROWS = OUT_F // O_SHARDS              # 512 out-features per core
N_IC = IN_F // 128                    # 16 contraction chunks
N_TG = TOK // 512                     # 16 token groups of 512
N_OS = ROWS // 128                    # 4 output-row subtiles
SCALES = [2, 4, 8, 16, 32, 64]        # alphas idx 1..6; idx 0 = identity

bf16 = mybir.dt.bfloat16
f32 = mybir.dt.float32
ADD = mybir.AluOpType.add
MULT = mybir.AluOpType.mult

_CACHE = {}


def _ap(t, part0, nparts, free_off, dims):
    """SBUF AP: partitions [part0, part0+nparts), free dims (stride,count)."""
    h = t.ap()
    fs = h.ap[0][0]  # per-partition free span
    return bass.AP(tensor=h.tensor, offset=part0 * fs + free_off,
                   ap=[[fs, nparts]] + [list(d) for d in dims])


def _build_nc(tok=TOK, debug_wflat=False):
    n_tg = tok // 512
    nc = bacc.Bacc("TRN2", target_bir_lowering=False, debug=False, num_devices=N_CORES)
    xT = nc.dram_tensor("xT", [IN_F, tok], bf16, kind="ExternalInput")
    wsb = nc.dram_tensor("wsb", [128, 8192], bf16, kind="ExternalInput")
    bias_s = nc.dram_tensor("bias_s", [1, ROWS], f32, kind="ExternalInput")
    alphas = nc.dram_tensor("alphas", [1, 7], f32, kind="ExternalInput")
    gumbels = nc.dram_tensor("gumbels", [1, 7], f32, kind="ExternalInput")
    out = nc.dram_tensor("out", [ROWS, tok], bf16, kind="ExternalOutput")
    if debug_wflat:
        w_flat = nc.dram_tensor("w_flat", [128, 8192], bf16, kind="ExternalOutput")
    else:
        w_flat = nc.dram_tensor("w_flat", [128, 8192], bf16)

    with TileContext(nc) as tc:
        # ---------- softmax(alphas + gumbels) broadcast to 128 partitions ----------
        asb = nc.alloc_sbuf_tensor("asb", [128, 7], f32).ap()
        gsb = nc.alloc_sbuf_tensor("gsb", [128, 7], f32).ap()
        a_bc = nc.alloc_sbuf_tensor("a_bc", [128, 7], f32).ap()
        ssum = nc.alloc_sbuf_tensor("ssum", [128, 1], f32).ap()
        nc.gpsimd.dma_start(out=asb, in_=bass.AP(tensor=alphas, offset=0, ap=[[0, 128], [1, 7]]))
        nc.gpsimd.dma_start(out=gsb, in_=bass.AP(tensor=gumbels, offset=0, ap=[[0, 128], [1, 7]]))
        nc.vector.tensor_tensor(out=asb, in0=asb, in1=gsb, op=ADD)
        nc.scalar.activation(out=asb, in_=asb, func=mybir.ActivationFunctionType.Exp)
        nc.vector.tensor_reduce(out=ssum, in_=asb, axis=mybir.AxisListType.X, op=ADD)
        nc.vector.reciprocal(out=ssum, in_=ssum)
        nc.vector.tensor_scalar_mul(a_bc, asb, ssum)

        # ---------- bias: [1, 512] -> [128 part, 4] (per-osub per-partition) ----------
        bias_sb = nc.alloc_sbuf_tensor("bias_sb", [128, N_OS], f32).ap()
        with nc.allow_non_contiguous_dma(reason="512-element one-time bias transpose"):
            nc.gpsimd.dma_start(out=bias_sb, in_=bass.AP(tensor=bias_s, offset=0, ap=[[1, 128], [128, N_OS]]))

        # ---------- load W chunk in build layout ----------
        # partition pi = P*8 + q (P in-col pair, q out 64-block); free (r, phat, s)
        wb_t = nc.alloc_sbuf_tensor("wb", [128, 8192], bf16)
        wb = wb_t.ap()
        nc.sync.dma_start(out=wb, in_=wsb.ap())

        # ---------- pyramid buffers ----------
        # S/T scale b: free off(u_ext, v, k) = u_ext*64 + v*b + k, size 8192/b
        S, T = {}, {}
        for b in SCALES:
            S[b] = nc.alloc_sbuf_tensor(f"S{b}", [128, 8192 // b], bf16)
            T[b] = nc.alloc_sbuf_tensor(f"T{b}", [128, 8192 // b], bf16)
        # Ghat scale b: off(u_ext, v, khat) = u_ext*128 + v*2b + khat, size 16384/b
        G = {}
        for b in SCALES:
            G[b] = nc.alloc_sbuf_tensor(f"G{b}", [128, 16384 // b], bf16)
        tmp_t = nc.alloc_sbuf_tensor("tmpu", [128, 2048], bf16)
        d_t = nc.alloc_sbuf_tensor("dbuf", [128, 4096], bf16)
        t2_t = nc.alloc_sbuf_tensor("t2buf", [128, 4096], bf16)
        acc_t = nc.alloc_sbuf_tensor("acc", [128, 8192], bf16)

        def rd(t, b, du, dv, k0, kn):
            # read scale-b S/T array over parent raster (U, phat+V merged, k)
            return _ap(t.tensor if isinstance(t, bass.AP) else t, 0, 128,
                       128 * du + b * dv + k0,
                       [[256, 32 // b], [2 * b, 64 // b], [1, kn]])

        def wr(t, B, h):
            # write scale-B S/T half h (k in [h*b, h*b+b)), b = B//2
            b = B // 2
            return _ap(t, 0, 128, h * b, [[128, 32 // b], [2 * b, 64 // b], [1, b]])

        def tmpap(b):
            return _ap(tmp_t, 0, 128, 0, [[64, 32 // b], [b, 64 // b], [1, b]])

        TTv = nc.vector.tensor_tensor
        CPs = nc.scalar.copy
        CPv = nc.vector.tensor_copy

        # ---------- up-sweep ----------
        # level 1 -> 2 (children are wb; T_1 = 0)
        nc.vector.memset(T[2].ap(), 0.0)
        TTv(out=wr(S[2], 2, 0), in0=rd(wb_t, 1, 0, 0, 0, 1), in1=rd(wb_t, 1, 1, 1, 0, 1), op=ADD)
        CPs(out=wr(S[2], 2, 1), in_=rd(wb_t, 1, 0, 1, 0, 1))
        CPv(out=wr(T[2], 2, 1), in_=rd(wb_t, 1, 1, 0, 0, 1))
        # levels b -> B
        for b, B in zip(SCALES[:-1], SCALES[1:]):
            TTv(out=tmpap(b), in0=rd(S[b], b, 0, 0, 0, b), in1=rd(S[b], b, 1, 1, 0, b), op=ADD)
            TTv(out=wr(S[B], B, 0), in0=tmpap(b), in1=rd(T[b], b, 0, 1, 0, b), op=ADD)
            CPs(out=wr(S[B], B, 1), in_=rd(S[b], b, 0, 1, 0, b))
            CPs(out=wr(T[B], B, 0), in_=rd(T[b], b, 1, 0, 0, b))
            TTv(out=tmpap(b), in0=rd(T[b], b, 0, 0, 0, b), in1=rd(S[b], b, 1, 0, 0, b), op=ADD)
            TTv(out=wr(T[B], B, 1), in0=tmpap(b), in1=rd(T[b], b, 1, 1, 0, b), op=ADD)

        # ---------- down-sweep (signed-lag Ghat pyramid) ----------
        def dfull(t, b):
            return _ap(t, 0, 128, 0, [[1, 8192 // b]])

        # Ghat_64 init: G64[phat, 0, lam+64] = t2_64[lam mod 64]
        TTv(out=dfull(d_t, 64), in0=dfull(S[64], 64), in1=dfull(T[64], 64), op=ADD)
        nc.vector.tensor_scalar(out=dfull(t2_t, 64), in0=dfull(d_t, 64),
                                scalar1=a_bc[:, 6:7], scalar2=1.0 / 64,
                                op0=MULT, op1=MULT)
        CPv(out=_ap(G[64], 0, 128, 64, [[128, 2], [1, 64]]),
            in_=_ap(t2_t, 0, 128, 0, [[64, 2], [1, 64]]))
        CPs(out=_ap(G[64], 0, 128, 1, [[128, 2], [1, 63]]),
            in_=_ap(t2_t, 0, 128, 1, [[64, 2], [1, 63]]))

        for bi in range(len(SCALES) - 2, -1, -1):
            b, B = SCALES[bi], SCALES[bi + 1]
            idx = bi + 1
            TTv(out=dfull(d_t, b), in0=dfull(S[b], b), in1=dfull(T[b], b), op=ADD)
            nc.vector.tensor_scalar(out=dfull(t2_t, b), in0=dfull(d_t, b),
                                    scalar1=a_bc[:, idx:idx + 1], scalar2=1.0 / b,
                                    op0=MULT, op1=MULT)
            for eu in range(2):
                for ev in range(2):
                    for sgn in range(2):
                        # sgn 0: lam in [0, b) (cnt b, k0 = 0)
                        # sgn 1: lam in [-(b-1), -1] (cnt b-1, k0 = 1)
                        cnt = b if sgn == 0 else b - 1
                        if cnt == 0:
                            continue
                        lam0 = 0 if sgn == 0 else -(b - 1)
                        k0 = lam0 % b  # t2 k index at lam0
                        oap = _ap(G[b], 0, 128,
                                  256 * eu + 2 * b * ev + b + lam0,
                                  [[512, 32 // b], [4 * b, 64 // b], [1, cnt]])
                        t2ap = _ap(t2_t, 0, 128,
                                   128 * eu + b * ev + k0,
                                   [[256, 32 // b], [2 * b, 64 // b], [1, cnt]])
                        gbap = _ap(G[B], 0, 128,
                                   lam0 + b * (ev - eu) + B,
                                   [[256, 32 // b], [4 * b, 64 // b], [1, cnt]])
                        TTv(out=oap, in0=t2ap, in1=gbap, op=ADD)

        # ---------- final: acc = a0*W + G2[r2, s2, (s%2 - r%2) + 2] ----------
        nc.vector.tensor_scalar_mul(dfull(acc_t, 1), dfull(wb_t, 1), a_bc[:, 0:1])
        for er in range(2):
            for es in range(2):
                oap = _ap(acc_t, 0, 128, 128 * er + es, [[256, 32], [2, 64]])
                gap = _ap(G[2], 0, 128, (es - er) + 2, [[256, 32], [4, 64]])
                TTv(out=oap, in0=oap, in1=gap, op=ADD)

        # ---------- store flat + transposed reload, 4 pipelined groups ----------
        wT = [nc.alloc_sbuf_tensor(f"wT{ic}", [128, ROWS], bf16).ap() for ic in range(N_IC)]
        for g in range(4):
            nc.sync.dma_start(
                out=bass.AP(tensor=w_flat, offset=g * 32 * 8192, ap=[[8192, 32], [1, 8192]]),
                in_=acc_t.ap()[32 * g:32 * g + 32, :])
            for ic in range(4 * g, 4 * g + 4):
                nc.sync.dma_start(
                    out=wT[ic],
                    in_=bass.AP(tensor=w_flat, offset=ic * 65536, ap=[[128, 512], [1, 128]]),
                    transpose=True)

        # ---------- main matmul over token groups ----------
        with (
            tc.tile_pool(name="xt", bufs=2) as xt_pool,
            tc.tile_pool(name="psum", bufs=8, space="PSUM") as psum_pool,
            tc.tile_pool(name="osb", bufs=8) as osb_pool,
        ):
            for tg in range(n_tg):
                # one DMA loads all 16 ic-chunks for this token group
                xt = xt_pool.tile([128, N_IC * 512], bf16, name="xt")
                nc.sync.dma_start(
                    out=xt[:],
                    in_=bass.AP(tensor=xT, offset=tg * 512,
                                ap=[[tok, 128], [128 * tok, N_IC], [1, 512]]))
                psums = [psum_pool.tile([128, 512], f32, name=f"ps{o}", tag="ps")
                         for o in range(N_OS)]
                for ic in range(N_IC):
                    rhs = xt[:, ic * 512:(ic + 1) * 512]
                    for o in range(N_OS):
                        nc.tensor.matmul(psums[o][:], wT[ic][:, o * 128:(o + 1) * 128], rhs,
                                         start=(ic == 0), stop=(ic == N_IC - 1))
                for o in range(N_OS):
                    ot = osb_pool.tile([128, 512], bf16, name="ot")
                    nc.scalar.activation(out=ot[:], in_=psums[o][:],
                                         func=mybir.ActivationFunctionType.Identity,
                                         bias=bias_sb[:, o:o + 1], scale=1.0)
                    nc.sync.dma_start(out=out.ap()[o * 128:(o + 1) * 128, tg * 512:(tg + 1) * 512],
                                      in_=ot[:])

    nc.compile()
    return nc


def make_ws_build(weight_f32):
    """Per-o-shard build-layout weight: [128, 8192] bf16,
    [pi = P*8+q, r*128 + phat*64 + s] = W[o*512 + q*64 + r, P*128 + phat*64 + s]."""
    import ml_dtypes
    outs = []
    for o in range(O_SHARDS):
        Wo = weight_f32[o * ROWS:(o + 1) * ROWS]            # [512, 2048]
        t = Wo.reshape(8, 64, 16, 2, 64).transpose(2, 0, 1, 3, 4)  # (P,q,r,phat,s)
        outs.append(np.ascontiguousarray(t.reshape(128, 8192)).astype(ml_dtypes.bfloat16))
    return outs


def make_in_maps(x, weight, bias, alphas, gumbels, tok=TOK):
    import ml_dtypes
    t_sh = TOK_TOTAL // tok
    x2 = np.asarray(x, np.float32).reshape(TOK_TOTAL, IN_F)
    xTh = np.ascontiguousarray(x2.T).astype(ml_dtypes.bfloat16)   # [2048, 16384]
    xslices = [np.ascontiguousarray(xTh[:, t * tok:(t + 1) * tok]) for t in range(t_sh)]
    weight = np.asarray(weight, np.float32)
    bias = np.asarray(bias, np.float32)
    wslices = make_ws_build(weight)
    bslices = [np.ascontiguousarray(bias[o * ROWS:(o + 1) * ROWS]).reshape(1, ROWS)
               for o in range(O_SHARDS)]
    al = np.asarray(alphas, np.float32).reshape(1, 7)
    gu = np.asarray(gumbels, np.float32).reshape(1, 7)
    in_maps = []
    for c in range(N_CORES):
        t, o = divmod(c, O_SHARDS)
        in_maps.append({"xT": xslices[t % t_sh], "wsb": wslices[o], "bias_s": bslices[o],
                        "alphas": al, "gumbels": gu})
    return in_maps


def kernel(x, weight, bias, alphas, gumbels):
    if "nc" not in _CACHE:
        _CACHE["nc"] = _build_nc()
    nc = _CACHE["nc"]
    in_maps = make_in_maps(x, weight, bias, alphas, gumbels)
    res = run_bass_kernel_spmd(nc, in_maps, core_ids=list(range(N_CORES)))
    row_blocks = []
    for o in range(O_SHARDS):
        row_blocks.append(np.concatenate(
            [res.results[t * O_SHARDS + o]["out"] for t in range(T_SHARDS)], axis=1))
    full_t = np.concatenate(row_blocks, axis=0)              # [2048, 16384] bf16
    return np.ascontiguousarray(full_t.T).astype(np.float32).reshape(BATCH, TOKENS, OUT_F)
